# revision 3
# baseline (speedup 1.0000x reference)
"""LoraLinear (x @ W.T + 2*(x @ A.T) @ B.T) on 8 TRN2 NeuronCores — v3.

v2 + DMA-engine load rebalancing. Trace analysis of v2 showed SDMA
engine 15 services descriptors ~15% slower than engines 0-14 (a known
TRN2 quirk); every chunk-completion semaphore waits on engine 15's
last-place finish, stretching the input stream ~3.5 us. Per the HWDGE
port map (engine = bits[4:2]<<1 | bit[6] of the partition index),
engine 15 owns partitions {92-95, 124-127}. v3 re-tiles the contraction
dim with variable-width tiles — 25x128 + 4x[0:120) + 2x[0:108) main
tiles and per-block tails of [0:108) + [0:92) — so partitions 92+ carry
~11% fewer bytes and engine 15 finishes with the pack. A DMA to a
partition sub-range emits no data descriptors for absent engines but
still increments its semaphore by 16.

Everything else as v2: host-side u = 2*(x @ lora_A.T) (fp32-exact),
fp16 x fp8 PE with fp32 PSUM, 2x column tiling (even tiles -> PSUM rows
0-63, odd -> 64-127), single packed byte blob (xt then W main then
per-block tails), staggered block closes with early DVE casts, b0-2
shipped by sync gated on the last input chunk, b3 cast+shipped by ACT,
host adds the two PSUM halves.

Self-contained: shapes hardcoded for
  x [64, 4096] f32, weight [16384, 4096] f32,
  lora_A [64, 4096] f32, lora_B [16384, 64] f32  ->  out [64, 16384] f32
"""

import numpy as np

import concourse.bass as bass
import concourse.mybir as mybir
from concourse.bass_utils import run_bass_kernel_spmd

N_CORES = 8
TOK = 64          # tokens
IN_F = 4096       # in_features (contraction)
OUT_F = 16384     # out_features
R = 64            # lora rank
SCALING = 2.0
O_SHARD = OUT_F // N_CORES   # 2048 out features per core
P = 128
NB = O_SHARD // 512          # 4 psum blocks of 512
F16 = mybir.dt.float16
F32 = mybir.dt.float32
F8 = mybir.dt.float8e4
U8 = mybir.dt.uint8
WSCALE = 64.0                # W pre-scale folded into x (2^6)

# Variable-width contraction tiles. widths[t] = partitions [0:w) used by
# tile t; even tile index -> PE columns 0-63 / PSUM rows 0-63, odd ->
# columns 64-127 / rows 64-127. Tiles 0..30 are "main" (all 2048 cols);
# tiles 31 (odd half) and 32 (even half) are the per-block tails.
WIDTHS = [128] * 25 + [120] * 4 + [108] * 2 + [108, 92]
N_TILES = len(WIDTHS)                    # 33
N_MAIN = 31
assert sum(WIDTHS) == IN_F
VOFF = np.concatenate([[0], np.cumsum(WIDTHS)])   # contraction offsets

XT_B = N_TILES * TOK * 2                 # 4224: stationary region bytes
W0_B = XT_B                              # W main region base
TAIL_B = W0_B + N_MAIN * O_SHARD         # per-block tails base
BLOB_B = TAIL_B + NB * 1024              # 1024 = 2 tail tiles x 512 B

# sync-ring chunks: (byte_lo, byte_hi, width, first_tile) — rectangles
# over partitions [0:width). Tails are separate per-block dmas below.
MAIN_CHUNKS = [
    (0, W0_B + 4 * O_SHARD, 128, 0),                       # xt + fat 0-3
    (W0_B + 4 * O_SHARD, W0_B + 9 * O_SHARD, 128, 4),      # fat 4-8
    (W0_B + 9 * O_SHARD, W0_B + 14 * O_SHARD, 128, 9),     # fat 9-13
    (W0_B + 14 * O_SHARD, W0_B + 19 * O_SHARD, 128, 14),   # fat 14-18
    (W0_B + 19 * O_SHARD, W0_B + 25 * O_SHARD, 128, 19),   # fat 19-24
    (W0_B + 25 * O_SHARD, W0_B + 29 * O_SHARD, 120, 25),   # thin120 25-28
    (W0_B + 29 * O_SHARD, W0_B + 31 * O_SHARD, 108, 29),   # thin108 29-30
]
TILE_CHUNK = {}                          # first_tile -> chunk idx
for ci, (_, _, _, t0) in enumerate(MAIN_CHUNKS):
    TILE_CHUNK[t0] = ci
N_WARM_MM = 18               # PE warm-up dummies: cover preamble->C0 wait

BU_B = TOK * 2 + O_SHARD * 2             # 4224: ut row (128B) + bt row


def _build_nc():
    nc = bass.Bass()
    blob = nc.dram_tensor("blob", [P, BLOB_B], U8, kind="ExternalInput")
    bu = nc.dram_tensor("bu", [R, BU_B], U8, kind="ExternalInput")
    out2 = nc.dram_tensor("out2", [2 * TOK, O_SHARD], F16, kind="ExternalOutput")

    with (
        nc.sbuf_tensor("blob_sb", [P, BLOB_B], U8) as blob_sb,
        nc.sbuf_tensor("bu_sb", [R, BU_B], U8) as bu_sb,
        nc.sbuf_tensor("out_sb", [2 * TOK, O_SHARD], F16) as out_sb,
        nc.sbuf_tensor("warm_sb", [1, 8], F16) as warm_sb,
        nc.psum_tensor("ps_o", [2 * TOK, NB, 512], F32) as ps_o,
        nc.psum_tensor("ps_warm", [TOK, 512], F32) as ps_warm,
        nc.semaphore("b_sem") as b_sem,       # bu DMA done at >= 16
        nc.semaphore("pe_sem") as pe_sem,     # block stop-matmuls done (+1)
        nc.semaphore("cpv_sem") as cpv_sem,   # DVE casts done (+1)
        nc.semaphore("cps_sem") as cps_sem,   # ACT cast done (+1)
        nc.semaphore("done_sem") as done_sem, # out DMA done (+16 each)
        nc.Block(no_gpsimd_drain=True) as block,
    ):
        c_sems = [nc.alloc_semaphore(f"c_sem{ci}")
                  for ci in range(len(MAIN_CHUNKS))]
        # per-block tail sems: t31 (odd half) and t32 (even half)
        t_sems = [[nc.alloc_semaphore(f"t_sem{b}_{j}") for j in range(2)]
                  for b in range(NB)]

        def xt_v(t):
            # stationary (x/64).T fp16 for tile t: [w, 64]
            w = WIDTHS[t]
            return blob_sb[0:w, t * 128:(t + 1) * 128].bitcast(F16)

        def w_v(t, b):
            # moving W fp8 [w, 512] for tile t, block b
            w = WIDTHS[t]
            if t < N_MAIN:
                off = W0_B + t * O_SHARD + b * 512
            else:
                off = TAIL_B + b * 1024 + (t - N_MAIN) * 512
            return blob_sb[0:w, off:off + 512].bitcast(F8)

        ut_v = bu_sb[:, 0:TOK * 2].bitcast(F16)              # [64, 64]

        def bt_v(b):
            off = TOK * 2 + b * 1024
            return bu_sb[:, off:off + 1024].bitcast(F16)     # [64, 512]

        @block.sync
        def _(sync):
            for ci, (lo, hi, w, _) in enumerate(MAIN_CHUNKS):
                sync.dma_start(
                    out=blob_sb[0:w, lo:hi], in_=blob[0:w, lo:hi],
                ).then_inc(c_sems[ci], 16)
            for b in range(NB):
                for j, w in enumerate((108, 92)):
                    lo = TAIL_B + b * 1024 + j * 512
                    sync.dma_start(
                        out=blob_sb[0:w, lo:lo + 512],
                        in_=blob[0:w, lo:lo + 512],
                    ).then_inc(t_sems[b][j], 16)
            # ship blocks 0-2 once the last input chunk is off the wire
            # and the DVE casts have landed
            sync.wait_ge(t_sems[NB - 1][1], 16)
            sync.wait_ge(cpv_sem, 3)
            sync.dma_start(out=out2[:, 0:1536],
                           in_=out_sb[:, 0:1536]).then_inc(done_sem, 16)
            sync.wait_ge(done_sem, 32)

        @block.tensor
        def _(tensor):
            def dummy_mm(n=1):
                # scratch matmul keeps the HAM activity window busy while
                # the PE waits on DMA; garbage input, never-read output.
                for _ in range(n):
                    nc.tensor.matmul(
                        ps_warm[:], xt_v(0), w_v(0, 0),
                        start=True, stop=True, tile_position=(0, 0))

            def main_tile(t):
                even = (t % 2 == 0)
                rows = ps_o[0:TOK] if even else ps_o[TOK:2 * TOK]
                pos = (0, 0) if even else (0, TOK)
                for b in range(NB):
                    nc.tensor.matmul(
                        rows[:, b, :], xt_v(t), w_v(t, b),
                        start=(t < 2), stop=False, tile_position=pos)

            dummy_mm(N_WARM_MM)                # HAM warm-up, no waits
            tensor.wait_ge(c_sems[0], 16)      # xt + fat tiles 0-3
            for t in range(4):
                main_tile(t)
            # lora term into the open k-even accumulation groups
            tensor.wait_ge(b_sem, 16)
            for b in range(NB):
                nc.tensor.matmul(
                    ps_o[0:TOK, b, :], ut_v, bt_v(b),
                    start=False, stop=False, tile_position=(0, 0))
            for t in range(4, N_MAIN):
                if t in TILE_CHUNK:
                    dummy_mm(1)                # fill the DMA-wait gap
                    tensor.wait_ge(c_sems[TILE_CHUNK[t]], 16)
                main_tile(t)
            # per-block tails: close each block as its ~100 KiB lands
            for b in range(NB):
                dummy_mm(1)
                tensor.wait_ge(t_sems[b][0], 16)
                nc.tensor.matmul(              # tile 31: odd-half stop
                    ps_o[TOK:2 * TOK, b, :], xt_v(31), w_v(31, b),
                    start=False, stop=True, tile_position=(0, TOK))
                tensor.wait_ge(t_sems[b][1], 16)
                nc.tensor.matmul(              # tile 32: even-half stop
                    ps_o[0:TOK, b, :], xt_v(32), w_v(32, b),
                    start=False, stop=True, tile_position=(0, 0),
                ).then_inc(pe_sem, 1)

        @block.vector
        def _(vector):
            for b in range(3):
                vector.wait_ge(pe_sem, b + 1)  # block b stop-matmuls done
                nc.vector.tensor_copy(
                    out=out_sb[:, b * 512:(b + 1) * 512],
                    in_=ps_o[:, b, :]).then_inc(cpv_sem, 1)

        @block.scalar
        def _(scalar):
            # ut+bt load rides the ACT HWDGE queue, desc-gen parallel to
            # the sync queue's blob chunks
            scalar.dma_start(out=bu_sb[:], in_=bu[:]).then_inc(b_sem, 16)
            # dummy 1-elem copy pre-loads the ACT function table (~1.3 us)
            # during the DMA phase instead of in the drain tail.
            nc.scalar.copy(out=warm_sb[:], in_=warm_sb[:])
            scalar.wait_ge(pe_sem, 4)          # block 3 stop-matmuls done
            nc.scalar.copy(
                out=out_sb[:, 1536:2048], in_=ps_o[:, 3, :]).then_inc(cps_sem, 1)
            scalar.wait_ge(cps_sem, 1)
            scalar.dma_start(out=out2[:, 1536:2048],
                             in_=out_sb[:, 1536:2048]).then_inc(done_sem, 16)

    return nc


_NC_CACHE = None


def _get_nc():
    global _NC_CACHE
    if _NC_CACHE is None:
        _NC_CACHE = _build_nc()
    return _NC_CACHE


def _prep_in_maps(x, weight, lora_A, lora_B):
    f8 = mybir.dt.np(F8)
    xt_full = (x / WSCALE).T.astype(np.float16)       # [4096, 64]
    # stationary region: tile t's x rows in partitions [0:w), 128B cols
    xt_b = np.zeros((P, XT_B), np.uint8)
    for t, w in enumerate(WIDTHS):
        rows = np.ascontiguousarray(xt_full[VOFF[t]:VOFF[t] + w])
        xt_b[0:w, t * 128:(t + 1) * 128] = rows.view(np.uint8)
    ut = (SCALING * (lora_A @ x.T)).astype(np.float16)        # [64, 64]
    wt_full = weight.T * WSCALE                       # [4096, 16384]
    bt_full = lora_B.T.astype(np.float16)             # [64, 16384]
    in_maps = []
    for c in range(N_CORES):
        sl = slice(c * O_SHARD, (c + 1) * O_SHARD)
        wt8 = wt_full[:, sl].astype(f8)               # [4096, 2048] fp8
        blob = np.zeros((P, BLOB_B), np.uint8)
        blob[:, :XT_B] = xt_b
        for t in range(N_MAIN):
            w = WIDTHS[t]
            blob[0:w, W0_B + t * O_SHARD: W0_B + (t + 1) * O_SHARD] = \
                np.ascontiguousarray(wt8[VOFF[t]:VOFF[t] + w]).view(np.uint8)
        for b in range(NB):
            for j, t in enumerate((31, 32)):
                w = WIDTHS[t]
                lo = TAIL_B + b * 1024 + j * 512
                blob[0:w, lo:lo + 512] = np.ascontiguousarray(
                    wt8[VOFF[t]:VOFF[t] + w, b * 512:(b + 1) * 512]).view(np.uint8)
        bu = np.ascontiguousarray(np.concatenate(
            [ut.view(np.uint8),
             np.ascontiguousarray(bt_full[:, sl]).view(np.uint8)], axis=1))
        in_maps.append({"blob": blob, "bu": bu})
    return in_maps


def kernel(x, weight, lora_A, lora_B, trace=False):
    x = np.asarray(x, dtype=np.float32)
    weight = np.asarray(weight, dtype=np.float32)
    lora_A = np.asarray(lora_A, dtype=np.float32)
    lora_B = np.asarray(lora_B, dtype=np.float32)
    nc = _get_nc()
    in_maps = _prep_in_maps(x, weight, lora_A, lora_B)
    res = run_bass_kernel_spmd(nc, in_maps, core_ids=list(range(N_CORES)),
                               trace=trace)
    # each core returns [128, 2048]: rows 0-63 even-tile partial (+ lora),
    # rows 64-127 odd-tile partial; the halves sum to the full result.
    out = np.concatenate(
        [np.asarray(res.results[c]["out2"], dtype=np.float32)
         for c in range(N_CORES)], axis=1)
    out = out[:TOK] + out[TOK:]
    if trace:
        kernel.last_results = res
    return out
